# revision 4
# baseline (speedup 1.0000x reference)
"""Sparse biased attention kernel (nn_CustomModel_60851096649964) on TRN2.

Contract: kernel(**inputs) takes FULL unsharded numpy inputs (keyed as in
setup_inputs()) and returns the FULL [2880, 10, 128] float32 output.

Data-parallel over bs*T=2880 across 8 NeuronCores (nb=360 each), groups of
g=6 batch elements; 128-d weights replicated. Key restructurings vs a naive
per-batch implementation:

  - K-projection folded away by associativity:
      S^T = (m @ Wk @ bdq)^T-chunks = mT_chunk^T @ (Wk @ bdq), so one
      [128,480] matmul (qk = Wk @ blockdiag(q/4)) replaces per-batch
      K-projections; score matmuls use mT chunks as stationary (shared
      with the V-projection).
  - Bias add folded into exp via host-precomputed exp(bias):
      exp(S + B) = exp(S) * expB1, with expB1 = 0 exactly at masked slots
      (rel == -1) and at kv-padding — no PE bias matmuls, no -inf handling.
  - W_cross_proj folded into W_cattn on host (Wcc = Wcp @ Wcat): the
      intermediate y is never materialized; self-attn q2/k2/v2 project
      directly from the cross-attn merged output mT2.
  - Softmax division folded into the head-merge matmul: the merge selector
      Qsel is scaled per-(head,query) by 1/denominator (tensor_scalar),
      so normalization costs one [80,10] op instead of [80,128].
  - V token-major via mT-chunk-stationary matmuls (out = mT_c^T @ Wv);
      denominators from a persistent ones-column in the vn tiles.
      (NB: dma_start_transpose silently corrupts data for free dims > 128
      or strided outputs — do not use it for batched transposes.)
  - PSUM pools sized to exactly 8 banks (big 3 / ev 2 / small 3); input
      DMAs prefetched 3 groups ahead on the SP queue so they never queue
      behind the output store; gpsimd (Pool) never touches PSUM (ISA).

All matmul I/O is bf16 (fp32 PSUM accumulation); rel-err ~6.5e-3 vs the
fp32 reference (budget 2e-2). TimelineSim estimate ~820 us/core vs the
previous version's ~1125 us.
"""

import numpy as np
import ml_dtypes

N_HEADS = 8
DIM = 128
HD = DIM // N_HEADS
BT, A, M = 2880, 10, 350
N_CORES = 8
NB = BT // N_CORES
G = 6
MPAD = 384
NEG = -60.0
BF = ml_dtypes.bfloat16

_PROGRAM_CACHE = {}


def _build_program(nb=NB, g=G, st_bufs=4, io_bufs=3, sb_bufs=2, mini_bufs=2, vt_bufs=2):
    from contextlib import ExitStack
    import concourse.bacc as bacc
    import concourse.mybir as mybir
    import concourse.tile as tile

    dt = mybir.dt
    BF16, F32 = dt.bfloat16, dt.float32
    EXP = mybir.ActivationFunctionType.Exp
    MULT = mybir.AluOpType.mult
    DIV = mybir.AluOpType.divide
    ng = nb // g
    assert nb % g == 0
    GA = g * A          # 60
    GH = g * 80         # 480
    GM = g * MPAD       # 2304

    nc = bacc.Bacc("TRN2", target_bir_lowering=False, debug=False)

    d_mT = nc.dram_tensor("mT", [128, nb * MPAD], BF16, kind="ExternalInput")
    d_aT = nc.dram_tensor("aT", [128, nb * A], BF16, kind="ExternalInput")
    d_eB1 = nc.dram_tensor("eB1", [128, nb * 3 * 80], BF16, kind="ExternalInput")
    d_eB2 = nc.dram_tensor("eB2", [10, nb * 80], BF16, kind="ExternalInput")
    d_w = {}
    for wname, shp in [("Wq4", [128, 128]), ("WkT", [128, 128]), ("Wv", [128, 128]),
                       ("Wccq", [128, 128]), ("WckT", [128, 128]), ("Wccv", [128, 128]),
                       ("Wsp", [128, 128]), ("QselG", [80, g * A]),
                       ("I128", [128, 128]),
                       ("Hmask", [80, 128]), ("BDmask", [128, 80])]:
        d_w[wname] = nc.dram_tensor(wname, shp, BF16, kind="ExternalInput")
    d_out = nc.dram_tensor("outT", [128, nb * A], F32, kind="ExternalOutput")

    with ExitStack() as ctx:
        tc = ctx.enter_context(tile.TileContext(nc))
        consts = ctx.enter_context(tc.tile_pool(name="consts", bufs=1))
        io = ctx.enter_context(tc.tile_pool(name="io", bufs=io_bufs))
        sb = ctx.enter_context(tc.tile_pool(name="sb", bufs=sb_bufs))
        pp_big = ctx.enter_context(tc.tile_pool(name="pp_big", bufs=st_bufs, space="PSUM"))
        pp_ev = ctx.enter_context(tc.tile_pool(name="pp_ev", bufs=vt_bufs, space="PSUM"))
        pp_small = ctx.enter_context(tc.tile_pool(name="pp_small", bufs=mini_bufs, space="PSUM"))

        w = {}
        for wname in d_w:
            t = consts.tile(list(d_w[wname].shape), BF16, tag=wname, name=wname)
            nc.sync.dma_start(t[:], d_w[wname].ap())
            w[wname] = t

        # persistent double-buffered tiles (ones cols / zero padding written once)
        vn2 = [consts.tile([128, g * 3, 129], BF16, tag=f"vn{i}", name=f"vn{i}") for i in range(2)]
        v2n2 = [consts.tile([128, g, 129], BF16, tag=f"v2n{i}", name=f"v2n{i}") for i in range(2)]
        for i in range(2):
            nc.gpsimd.memset(vn2[i][:, :, 128:129], 1.0)
            nc.gpsimd.memset(v2n2[i][:, :, 128:129], 1.0)

        def emit_inputs(j):
            bsj = j * g
            mTs = io.tile([128, GM], BF16, tag="mTs", name="mTs")
            nc.sync.dma_start(mTs[:], d_mT[:, bsj * MPAD:(bsj + g) * MPAD])
            aTs = io.tile([128, GA], BF16, tag="aTs", name="aTs")
            nc.sync.dma_start(aTs[:], d_aT[:, bsj * A:(bsj + g) * A])
            eB1s = io.tile([128, 3, GH], BF16, tag="eB1s", name="eB1s")
            nc.sync.dma_start(eB1s[:], d_eB1[:, bsj * 240:(bsj + g) * 240])
            eB2s = io.tile([10, GH], BF16, tag="eB2s", name="eB2s")
            nc.sync.dma_start(eB2s[:], d_eB2[:, bsj * 80:(bsj + g) * 80])
            return (mTs, aTs, eB1s, eB2s)

        PF = io_bufs
        pending = {}
        for j in range(min(PF, ng)):
            pending[j] = emit_inputs(j)

        for gi in range(ng):
            bs = gi * g
            vn = vn2[gi % 2]
            v2n = v2n2[gi % 2]
            mTs, aTs, eB1s, eB2s = pending.pop(gi)
            if gi + PF < ng:
                pending[gi + PF] = emit_inputs(gi + PF)

            # ---- cross attn: Q path ----
            small = pp_small.tile([128, 300], F32, tag="small", name="small")
            qp_ps = small[:, 0:GA]
            nc.tensor.matmul(qp_ps, w["Wq4"][:], aTs[:], start=True, stop=True)
            qpsb = sb.tile([128, GA], BF16, tag="qpsb")
            nc.vector.tensor_copy(qpsb[:], qp_ps)
            bdq = sb.tile([128, GH], BF16, tag="bdq")
            nc.vector.tensor_copy(
                bdq[:].rearrange("p (b h q) -> p b h q", b=g, h=N_HEADS),
                qpsb[:].rearrange("p (b q) -> p b q", b=g)
                    .unsqueeze(2).broadcast_to([128, g, N_HEADS, A]))
            nc.gpsimd.tensor_tensor(
                bdq[:].rearrange("p (b c) -> p b c", b=g),
                bdq[:].rearrange("p (b c) -> p b c", b=g),
                w["BDmask"][:].unsqueeze(1).broadcast_to([128, g, 80]), MULT)
            qk_ps = pp_big.tile([128, GH], F32, tag="big", name="qk_ps")[:]
            nc.tensor.matmul(qk_ps, w["WkT"][:], bdq[:], start=True, stop=True)
            qksb = sb.tile([128, GH], BF16, tag="qksb")
            nc.scalar.copy(qksb[:], qk_ps)

            # ---- V path: per-b token-major projection (mTs chunk stationary) ----
            for b in range(g):
                vn_ps = pp_big.tile([128, 384], F32, tag="big", name="vn_ps")
                for c in range(3):
                    nc.tensor.matmul(
                        vn_ps[:, c * 128:(c + 1) * 128],
                        mTs[:, (b * 3 + c) * 128:(b * 3 + c + 1) * 128],
                        w["Wv"][:], start=True, stop=True)
                if b % 2 == 0:
                    nc.vector.tensor_copy(
                        vn[:, b * 3:(b + 1) * 3, 0:128],
                        vn_ps[:].rearrange("p (a c) -> p a c", c=128))
                else:
                    nc.scalar.copy(
                        vn[:, b * 3:(b + 1) * 3, 0:128],
                        vn_ps[:].rearrange("p (a c) -> p a c", c=128))

            # ---- scores + exp ----
            eTsb = sb.tile([128, 3, GH], BF16, tag="eTsb")
            for c in range(3):
                stc = pp_big.tile([128, GH], F32, tag="big", name="stc")
                for b in range(g):
                    nc.tensor.matmul(
                        stc[:, b * 80:(b + 1) * 80],
                        mTs[:, (b * 3 + c) * 128:(b * 3 + c + 1) * 128],
                        qksb[:, b * 80:(b + 1) * 80], start=True, stop=True)
                nc.scalar.activation(eTsb[:, c, :], stc[:], EXP)
                nc.vector.tensor_tensor(eTsb[:, c, :], eTsb[:, c, :],
                                        eB1s[:, c, :], MULT)

            # ---- EV + normalize + merge ----
            mevsb = sb.tile([80, g, 128], BF16, tag="mevsb")
            qselb = sb.tile([80, GA], BF16, tag="qselb")
            recip = sb.tile([80, g], F32, tag="recip")
            for half in range(2):
                ev_ps = pp_ev.tile([128, 3, 129], F32, tag="ev", name="ev_ps")
                for bi in range(3):
                    b = half * 3 + bi
                    for c in range(3):
                        nc.tensor.matmul(
                            ev_ps[0:80, bi, :], eTsb[:, c, b * 80:(b + 1) * 80],
                            vn[:, b * 3 + c, :], start=(c == 0), stop=(c == 2))
                    nc.vector.reciprocal(recip[:, b:b + 1], ev_ps[0:80, bi, 128:129])
                    nc.vector.tensor_tensor(mevsb[:, b, :], ev_ps[0:80, bi, 0:128],
                                            w["Hmask"][:], MULT)
                    nc.vector.tensor_scalar_mul(qselb[:, b * A:(b + 1) * A],
                                                w["QselG"][:, b * A:(b + 1) * A],
                                                recip[:, b:b + 1])
            mT2_ps = small[:, GA:2 * GA]
            for b in range(g):
                nc.tensor.matmul(mT2_ps[:, b * A:(b + 1) * A], mevsb[:, b, :],
                                 qselb[:, b * A:(b + 1) * A], start=True, stop=True)
            mT2sb = sb.tile([128, GA], BF16, tag="mT2sb")
            nc.vector.tensor_copy(mT2sb[:], mT2_ps)

            # ---- self attn ----
            q2_ps = small[:, 2 * GA:3 * GA]
            nc.tensor.matmul(q2_ps, w["Wccq"][:], mT2sb[:], start=True, stop=True)
            q2sb = sb.tile([128, GA], BF16, tag="q2sb")
            nc.vector.tensor_copy(q2sb[:], q2_ps)
            bdq2 = sb.tile([128, GH], BF16, tag="bdq2")
            nc.vector.tensor_copy(
                bdq2[:].rearrange("p (b h q) -> p b h q", b=g, h=N_HEADS),
                q2sb[:].rearrange("p (b q) -> p b q", b=g)
                    .unsqueeze(2).broadcast_to([128, g, N_HEADS, A]))
            nc.gpsimd.tensor_tensor(
                bdq2[:].rearrange("p (b c) -> p b c", b=g),
                bdq2[:].rearrange("p (b c) -> p b c", b=g),
                w["BDmask"][:].unsqueeze(1).broadcast_to([128, g, 80]), MULT)
            qk2_ps = pp_big.tile([128, GH], F32, tag="big", name="qk2_ps")[:]
            nc.tensor.matmul(qk2_ps, w["WckT"][:], bdq2[:], start=True, stop=True)
            qk2sb = sb.tile([128, GH], BF16, tag="qk2sb")
            nc.scalar.copy(qk2sb[:], qk2_ps)

            v2_ps = small[:, 3 * GA:4 * GA]
            nc.tensor.matmul(v2_ps, w["Wccv"][:], mT2sb[:], start=True, stop=True)
            v2sb = sb.tile([128, GA], BF16, tag="v2sb")
            nc.vector.tensor_copy(v2sb[:], v2_ps)
            for b in range(g):
                v2t_ps = pp_ev.tile([10, 128], BF16, tag="ev", name="v2t_ps")
                nc.tensor.transpose(v2t_ps[:], v2sb[:, b * A:(b + 1) * A],
                                    w["I128"][:])
                if b % 2 == 0:
                    nc.vector.tensor_copy(v2n[0:10, b, 0:128], v2t_ps[:])
                else:
                    nc.scalar.copy(v2n[0:10, b, 0:128], v2t_ps[:])

            s2_ps = pp_big.tile([10, GH], F32, tag="big", name="s2_ps")[:]
            for b in range(g):
                nc.tensor.matmul(s2_ps[:, b * 80:(b + 1) * 80],
                                 mT2sb[:, b * A:(b + 1) * A],
                                 qk2sb[:, b * 80:(b + 1) * 80], start=True, stop=True)
            e2sb = sb.tile([10, GH], BF16, tag="e2sb")
            nc.scalar.activation(e2sb[:], s2_ps, EXP)
            nc.vector.tensor_tensor(e2sb[:], e2sb[:], eB2s[:], MULT)

            mev2sb = sb.tile([80, g, 128], BF16, tag="mev2sb")
            qselb2 = sb.tile([80, GA], BF16, tag="qselb2")
            recip2 = sb.tile([80, g], F32, tag="recip2")
            for half in range(2):
                ev2_ps = pp_ev.tile([128, 3, 129], F32, tag="ev", name="ev2_ps")
                for bi in range(3):
                    b = half * 3 + bi
                    nc.tensor.matmul(ev2_ps[0:80, bi, :], e2sb[:, b * 80:(b + 1) * 80],
                                     v2n[0:10, b, :], start=True, stop=True)
                    nc.vector.reciprocal(recip2[:, b:b + 1], ev2_ps[0:80, bi, 128:129])
                    nc.vector.tensor_tensor(mev2sb[:, b, :], ev2_ps[0:80, bi, 0:128],
                                            w["Hmask"][:], MULT)
                    nc.vector.tensor_scalar_mul(qselb2[:, b * A:(b + 1) * A],
                                                w["QselG"][:, b * A:(b + 1) * A],
                                                recip2[:, b:b + 1])
            m2T_ps = small[:, 4 * GA:5 * GA]
            for b in range(g):
                nc.tensor.matmul(m2T_ps[:, b * A:(b + 1) * A], mev2sb[:, b, :],
                                 qselb2[:, b * A:(b + 1) * A], start=True, stop=True)
            m2Tsb = sb.tile([128, GA], BF16, tag="m2Tsb")
            nc.vector.tensor_copy(m2Tsb[:], m2T_ps)

            out_ps = small[:, 0:GA]
            nc.tensor.matmul(out_ps, w["Wsp"][:], m2Tsb[:], start=True, stop=True)
            outsb = sb.tile([128, GA], F32, tag="outsb")
            nc.vector.tensor_copy(outsb[:], out_ps)
            nc.sync.dma_start(d_out[:, bs * A:(bs + g) * A], outsb[:])

    nc.compile()
    return nc


def get_program(**kw):
    key = tuple(sorted(kw.items()))
    if key not in _PROGRAM_CACHE:
        _PROGRAM_CACHE[key] = _build_program(**kw)
    return _PROGRAM_CACHE[key]


def _host_prep(a_token, m_token, a2m_pe, a_pe, Wq, Wk, Wv, W_cross_proj,
               W_cattn, W_self_proj, a2m_relation, a_relation):
    f32 = np.float32
    B = BT

    # dense exp-biases
    safe1 = np.maximum(a2m_relation, 0)[..., None]
    bias1 = np.take_along_axis(a2m_pe, safe1, axis=2)           # [B,A,M,H]
    bias1 = np.where(a2m_relation[..., None] >= 0, bias1, f32(NEG))
    bias1 = bias1.transpose(0, 3, 1, 2).reshape(B, 80, M)       # [B,(h,q),kv]
    eb1 = np.zeros((B, 80, MPAD), dtype=BF)
    eb1[:, :, :M] = np.exp(bias1).astype(BF)
    # -> [128, B, 3, 80]
    eb1 = np.ascontiguousarray(
        eb1.reshape(B, 80, 3, 128).transpose(3, 0, 2, 1))

    safe2 = np.maximum(a_relation, 0)[..., None]
    bias2 = np.take_along_axis(a_pe, safe2, axis=2)
    bias2 = np.where(a_relation[..., None] >= 0, bias2, f32(NEG))
    eb2 = np.exp(bias2.transpose(0, 3, 1, 2).reshape(B, 80, A))
    eb2 = np.ascontiguousarray(eb2.transpose(2, 0, 1)).astype(BF)  # [10, B, 80]

    Wcc = (W_cross_proj.astype(f32) @ W_cattn.astype(f32))
    consts = {
        "Wq4": (Wq.astype(f32) / 4.0).astype(BF),
        "WkT": np.ascontiguousarray(Wk.T).astype(BF),
        "Wv": Wv.astype(BF),
        "Wccq": (Wcc[:, :128] / 4.0).astype(BF),
        "WckT": np.ascontiguousarray(Wcc[:, 128:256].T).astype(BF),
        "Wccv": Wcc[:, 256:384].astype(BF),
        "Wsp": W_self_proj.astype(BF),
        "QselG": np.tile(np.tile(np.eye(A, dtype=BF), (N_HEADS, 1)), (1, G)),
        "I128": np.eye(128, dtype=BF),
    }
    hmask = np.zeros((80, 128), dtype=BF)
    for h in range(N_HEADS):
        hmask[h * A:(h + 1) * A, h * HD:(h + 1) * HD] = 1
    consts["Hmask"] = hmask
    bdmask = np.zeros((128, 80), dtype=BF)
    for h in range(N_HEADS):
        bdmask[h * HD:(h + 1) * HD, h * A:(h + 1) * A] = 1
    consts["BDmask"] = bdmask

    nb = NB
    in_maps = []
    for c in range(N_CORES):
        lo, hi = c * nb, (c + 1) * nb
        im = dict(consts)
        mt = np.zeros((128, nb, MPAD), dtype=BF)
        mt[:, :, :M] = m_token[lo:hi].transpose(2, 0, 1).astype(BF)
        im["mT"] = mt.reshape(128, nb * MPAD)
        im["aT"] = np.ascontiguousarray(
            a_token[lo:hi].transpose(2, 0, 1)).reshape(128, nb * A).astype(BF)
        # eB1 core slice: [128, nb, 3, 80] -> group-reorder [128, ng, 3, g, 80]
        e1 = eb1[:, lo:hi]                                    # [128, nb, 3, 80]
        e1 = e1.reshape(128, nb // G, G, 3, 80).transpose(0, 1, 3, 2, 4)
        im["eB1"] = np.ascontiguousarray(e1).reshape(128, nb * 240)
        im["eB2"] = np.ascontiguousarray(eb2[:, lo:hi]).reshape(10, nb * 80)
        in_maps.append(im)
    return in_maps


def kernel(a_token, m_token, a2m_pe, a_pe, Wq, Wk, Wv, W_cross_proj,
           W_cattn, W_self_proj, a2m_relation, a_relation):
    from concourse.bass_utils import run_bass_kernel_spmd

    nc = get_program()
    in_maps = _host_prep(a_token, m_token, a2m_pe, a_pe, Wq, Wk, Wv,
                         W_cross_proj, W_cattn, W_self_proj,
                         a2m_relation, a_relation)
    res = run_bass_kernel_spmd(nc, in_maps, list(range(N_CORES)))
    nb = NB
    out = np.empty((BT, A, DIM), dtype=np.float32)
    for c in range(N_CORES):
        out[c * nb:(c + 1) * nb] = res.results[c]["outT"].T.reshape(nb, A, DIM)
    return out


# revision 5
# speedup vs baseline: 2.1522x; 2.1522x over previous
"""Sparse biased attention kernel (nn_CustomModel_60851096649964) on TRN2.

Contract: kernel(**inputs) takes FULL unsharded numpy inputs (keyed as in
setup_inputs()) and returns the FULL [2880, 10, 128] float32 output.

Data-parallel over bs*T=2880 across 8 NeuronCores (nb=360 each), groups of
g=6 batch elements; 128-d weights replicated. Key restructurings vs a naive
per-batch implementation:

  - K-projection folded away by associativity:
      S^T = (m @ Wk @ bdq)^T-chunks = mT_chunk^T @ (Wk @ bdq), so one
      [128,480] matmul (qk = Wk @ blockdiag(q/4)) replaces per-batch
      K-projections; score matmuls use mT chunks as stationary (shared
      with the V-projection).
  - Bias add folded into exp via host-precomputed exp(bias):
      exp(S + B) = exp(S) * expB1, with expB1 = 0 exactly at masked slots
      (rel == -1) and at kv-padding — no PE bias matmuls, no -inf handling.
  - W_cross_proj folded into W_cattn on host (Wcc = Wcp @ Wcat): the
      intermediate y is never materialized; self-attn q2/k2/v2 project
      directly from the cross-attn merged output mT2.
  - Softmax division folded into the head-merge matmul: the merge selector
      Qsel is scaled per-(head,query) by 1/denominator (tensor_scalar),
      so normalization costs one [80,10] op instead of [80,128].
  - V token-major via mT-chunk-stationary matmuls (out = mT_c^T @ Wv);
      denominators from a persistent ones-column in the vn tiles.
      (NB: dma_start_transpose silently corrupts data for free dims > 128
      or strided outputs — do not use it for batched transposes.)
  - PSUM pools sized to exactly 8 banks (big 3 / ev 2 / small 3); input
      DMAs prefetched 3 groups ahead on the SP queue so they never queue
      behind the output store; gpsimd (Pool) never touches PSUM (ISA).

All matmul I/O is bf16 (fp32 PSUM accumulation); rel-err ~6.5e-3 vs the
fp32 reference (budget 2e-2). TimelineSim estimate ~820 us/core vs the
previous version's ~1125 us.
"""

import numpy as np
import ml_dtypes

N_HEADS = 8
DIM = 128
HD = DIM // N_HEADS
BT, A, M = 2880, 10, 350
N_CORES = 8
NB = BT // N_CORES
G = 6
MPAD = 384
NEG = -60.0
BF = ml_dtypes.bfloat16

_PROGRAM_CACHE = {}


def _build_program(nb=NB, g=G, st_bufs=4, io_bufs=4, sb_bufs=3, mini_bufs=2, vt_bufs=2):
    from contextlib import ExitStack
    import concourse.bacc as bacc
    import concourse.mybir as mybir
    import concourse.tile as tile

    dt = mybir.dt
    BF16, F32 = dt.bfloat16, dt.float32
    EXP = mybir.ActivationFunctionType.Exp
    MULT = mybir.AluOpType.mult
    DIV = mybir.AluOpType.divide
    ng = nb // g
    assert nb % g == 0
    GA = g * A          # 60
    GH = g * 80         # 480
    GM = g * MPAD       # 2304

    nc = bacc.Bacc("TRN2", target_bir_lowering=False, debug=False)

    d_mT = nc.dram_tensor("mT", [128, nb * MPAD], BF16, kind="ExternalInput")
    d_aT = nc.dram_tensor("aT", [128, nb * A], BF16, kind="ExternalInput")
    d_eB1 = nc.dram_tensor("eB1", [128, nb * 3 * 80], BF16, kind="ExternalInput")
    d_eB2 = nc.dram_tensor("eB2", [10, nb * 80], BF16, kind="ExternalInput")
    d_w = {}
    for wname, shp in [("Wq4", [128, 128]), ("WkT", [128, 128]), ("Wv", [128, 128]),
                       ("Wccq", [128, 128]), ("WckT", [128, 128]), ("Wccv", [128, 128]),
                       ("Wsp", [128, 128]), ("QselG", [80, g * A]),
                       ("I128", [128, 128]),
                       ("Hmask", [80, 128]), ("BDmask", [128, 80])]:
        d_w[wname] = nc.dram_tensor(wname, shp, BF16, kind="ExternalInput")
    d_out = nc.dram_tensor("outT", [128, nb * A], F32, kind="ExternalOutput")

    with ExitStack() as ctx:
        tc = ctx.enter_context(tile.TileContext(nc))
        consts = ctx.enter_context(tc.tile_pool(name="consts", bufs=1))
        io = ctx.enter_context(tc.tile_pool(name="io", bufs=io_bufs))
        sb = ctx.enter_context(tc.tile_pool(name="sb", bufs=sb_bufs))
        pp_big = ctx.enter_context(tc.tile_pool(name="pp_big", bufs=st_bufs, space="PSUM"))
        pp_ev = ctx.enter_context(tc.tile_pool(name="pp_ev", bufs=vt_bufs, space="PSUM"))
        pp_small = ctx.enter_context(tc.tile_pool(name="pp_small", bufs=mini_bufs, space="PSUM"))

        w = {}
        for wname in d_w:
            t = consts.tile(list(d_w[wname].shape), BF16, tag=wname, name=wname)
            nc.sync.dma_start(t[:], d_w[wname].ap())
            w[wname] = t

        # persistent double-buffered tiles (ones cols / zero padding written once)
        vn2 = [consts.tile([128, g * 3, 129], BF16, tag=f"vn{i}", name=f"vn{i}") for i in range(2)]
        v2n2 = [consts.tile([128, g, 129], BF16, tag=f"v2n{i}", name=f"v2n{i}") for i in range(2)]
        for i in range(2):
            nc.gpsimd.memset(vn2[i][:, :, 128:129], 1.0)
            nc.gpsimd.memset(v2n2[i][:, :, 128:129], 1.0)

        def emit_inputs(j):
            bsj = j * g
            mTs = io.tile([128, GM], BF16, tag="mTs", name="mTs")
            nc.sync.dma_start(mTs[:], d_mT[:, bsj * MPAD:(bsj + g) * MPAD])
            aTs = io.tile([128, GA], BF16, tag="aTs", name="aTs")
            nc.sync.dma_start(aTs[:], d_aT[:, bsj * A:(bsj + g) * A])
            eB1s = io.tile([128, 3, GH], BF16, tag="eB1s", name="eB1s")
            nc.sync.dma_start(eB1s[:], d_eB1[:, bsj * 240:(bsj + g) * 240])
            eB2s = io.tile([10, GH], BF16, tag="eB2s", name="eB2s")
            nc.sync.dma_start(eB2s[:], d_eB2[:, bsj * 80:(bsj + g) * 80])
            return (mTs, aTs, eB1s, eB2s)

        PF = io_bufs
        pending = {}
        for j in range(min(PF, ng)):
            pending[j] = emit_inputs(j)

        for gi in range(ng):
            bs = gi * g
            vn = vn2[gi % 2]
            v2n = v2n2[gi % 2]
            mTs, aTs, eB1s, eB2s = pending.pop(gi)
            if gi + PF < ng:
                pending[gi + PF] = emit_inputs(gi + PF)

            # ---- cross attn: Q path ----
            small = pp_small.tile([128, 300], F32, tag="small", name="small")
            qp_ps = small[:, 0:GA]
            nc.tensor.matmul(qp_ps, w["Wq4"][:], aTs[:], start=True, stop=True)
            qpsb = sb.tile([128, GA], BF16, tag="qpsb")
            nc.vector.tensor_copy(qpsb[:], qp_ps)
            bdq = sb.tile([128, GH], BF16, tag="bdq")
            nc.vector.tensor_copy(
                bdq[:].rearrange("p (b h q) -> p b h q", b=g, h=N_HEADS),
                qpsb[:].rearrange("p (b q) -> p b q", b=g)
                    .unsqueeze(2).broadcast_to([128, g, N_HEADS, A]))
            nc.gpsimd.tensor_tensor(
                bdq[:].rearrange("p (b c) -> p b c", b=g),
                bdq[:].rearrange("p (b c) -> p b c", b=g),
                w["BDmask"][:].unsqueeze(1).broadcast_to([128, g, 80]), MULT)
            qk_ps = pp_big.tile([128, GH], F32, tag="big", name="qk_ps")[:]
            nc.tensor.matmul(qk_ps, w["WkT"][:], bdq[:], start=True, stop=True)
            qksb = sb.tile([128, GH], BF16, tag="qksb")
            nc.scalar.copy(qksb[:], qk_ps)

            # ---- V path: per-b token-major projection (mTs chunk stationary) ----
            for b in range(g):
                vn_ps = pp_big.tile([128, 384], F32, tag="big", name="vn_ps")
                for c in range(3):
                    nc.tensor.matmul(
                        vn_ps[:, c * 128:(c + 1) * 128],
                        mTs[:, (b * 3 + c) * 128:(b * 3 + c + 1) * 128],
                        w["Wv"][:], start=True, stop=True)
                if b % 2 == 0:
                    nc.vector.tensor_copy(
                        vn[:, b * 3:(b + 1) * 3, 0:128],
                        vn_ps[:].rearrange("p (a c) -> p a c", c=128))
                else:
                    nc.scalar.copy(
                        vn[:, b * 3:(b + 1) * 3, 0:128],
                        vn_ps[:].rearrange("p (a c) -> p a c", c=128))

            # ---- scores + exp ----
            eTsb = sb.tile([128, 3, GH], BF16, tag="eTsb")
            for c in range(3):
                stc = pp_big.tile([128, GH], F32, tag="big", name="stc")
                for b in range(g):
                    nc.tensor.matmul(
                        stc[:, b * 80:(b + 1) * 80],
                        mTs[:, (b * 3 + c) * 128:(b * 3 + c + 1) * 128],
                        qksb[:, b * 80:(b + 1) * 80], start=True, stop=True)
                nc.scalar.activation(eTsb[:, c, :], stc[:], EXP)
                nc.vector.tensor_tensor(eTsb[:, c, :], eTsb[:, c, :],
                                        eB1s[:, c, :], MULT)

            # ---- EV + normalize + merge ----
            mevsb = sb.tile([80, g, 128], BF16, tag="mevsb")
            qselb = sb.tile([80, GA], BF16, tag="qselb")
            recip = sb.tile([80, g], F32, tag="recip")
            for half in range(2):
                ev_ps = pp_ev.tile([128, 3, 129], F32, tag="ev", name="ev_ps")
                for bi in range(3):
                    b = half * 3 + bi
                    for c in range(3):
                        nc.tensor.matmul(
                            ev_ps[0:80, bi, :], eTsb[:, c, b * 80:(b + 1) * 80],
                            vn[:, b * 3 + c, :], start=(c == 0), stop=(c == 2))
                    nc.vector.reciprocal(recip[:, b:b + 1], ev_ps[0:80, bi, 128:129])
                    nc.vector.tensor_tensor(mevsb[:, b, :], ev_ps[0:80, bi, 0:128],
                                            w["Hmask"][:], MULT)
                    nc.vector.tensor_scalar_mul(qselb[:, b * A:(b + 1) * A],
                                                w["QselG"][:, b * A:(b + 1) * A],
                                                recip[:, b:b + 1])
            mT2_ps = small[:, GA:2 * GA]
            for b in range(g):
                nc.tensor.matmul(mT2_ps[:, b * A:(b + 1) * A], mevsb[:, b, :],
                                 qselb[:, b * A:(b + 1) * A], start=True, stop=True)
            mT2sb = sb.tile([128, GA], BF16, tag="mT2sb")
            nc.vector.tensor_copy(mT2sb[:], mT2_ps)

            # ---- self attn ----
            q2_ps = small[:, 2 * GA:3 * GA]
            nc.tensor.matmul(q2_ps, w["Wccq"][:], mT2sb[:], start=True, stop=True)
            q2sb = sb.tile([128, GA], BF16, tag="q2sb")
            nc.vector.tensor_copy(q2sb[:], q2_ps)
            bdq2 = sb.tile([128, GH], BF16, tag="bdq2")
            nc.vector.tensor_copy(
                bdq2[:].rearrange("p (b h q) -> p b h q", b=g, h=N_HEADS),
                q2sb[:].rearrange("p (b q) -> p b q", b=g)
                    .unsqueeze(2).broadcast_to([128, g, N_HEADS, A]))
            nc.gpsimd.tensor_tensor(
                bdq2[:].rearrange("p (b c) -> p b c", b=g),
                bdq2[:].rearrange("p (b c) -> p b c", b=g),
                w["BDmask"][:].unsqueeze(1).broadcast_to([128, g, 80]), MULT)
            qk2_ps = pp_big.tile([128, GH], F32, tag="big", name="qk2_ps")[:]
            nc.tensor.matmul(qk2_ps, w["WckT"][:], bdq2[:], start=True, stop=True)
            qk2sb = sb.tile([128, GH], BF16, tag="qk2sb")
            nc.scalar.copy(qk2sb[:], qk2_ps)

            v2_ps = small[:, 3 * GA:4 * GA]
            nc.tensor.matmul(v2_ps, w["Wccv"][:], mT2sb[:], start=True, stop=True)
            v2sb = sb.tile([128, GA], BF16, tag="v2sb")
            nc.vector.tensor_copy(v2sb[:], v2_ps)
            for b in range(g):
                v2t_ps = pp_ev.tile([10, 128], BF16, tag="ev", name="v2t_ps")
                nc.tensor.transpose(v2t_ps[:], v2sb[:, b * A:(b + 1) * A],
                                    w["I128"][:])
                if b % 2 == 0:
                    nc.vector.tensor_copy(v2n[0:10, b, 0:128], v2t_ps[:])
                else:
                    nc.scalar.copy(v2n[0:10, b, 0:128], v2t_ps[:])

            s2_ps = pp_big.tile([10, GH], F32, tag="big", name="s2_ps")[:]
            for b in range(g):
                nc.tensor.matmul(s2_ps[:, b * 80:(b + 1) * 80],
                                 mT2sb[:, b * A:(b + 1) * A],
                                 qk2sb[:, b * 80:(b + 1) * 80], start=True, stop=True)
            e2sb = sb.tile([10, GH], BF16, tag="e2sb")
            nc.scalar.activation(e2sb[:], s2_ps, EXP)
            nc.vector.tensor_tensor(e2sb[:], e2sb[:], eB2s[:], MULT)

            mev2sb = sb.tile([80, g, 128], BF16, tag="mev2sb")
            qselb2 = sb.tile([80, GA], BF16, tag="qselb2")
            recip2 = sb.tile([80, g], F32, tag="recip2")
            for half in range(2):
                ev2_ps = pp_ev.tile([128, 3, 129], F32, tag="ev", name="ev2_ps")
                for bi in range(3):
                    b = half * 3 + bi
                    nc.tensor.matmul(ev2_ps[0:80, bi, :], e2sb[:, b * 80:(b + 1) * 80],
                                     v2n[0:10, b, :], start=True, stop=True)
                    nc.vector.reciprocal(recip2[:, b:b + 1], ev2_ps[0:80, bi, 128:129])
                    nc.vector.tensor_tensor(mev2sb[:, b, :], ev2_ps[0:80, bi, 0:128],
                                            w["Hmask"][:], MULT)
                    nc.vector.tensor_scalar_mul(qselb2[:, b * A:(b + 1) * A],
                                                w["QselG"][:, b * A:(b + 1) * A],
                                                recip2[:, b:b + 1])
            m2T_ps = small[:, 4 * GA:5 * GA]
            for b in range(g):
                nc.tensor.matmul(m2T_ps[:, b * A:(b + 1) * A], mev2sb[:, b, :],
                                 qselb2[:, b * A:(b + 1) * A], start=True, stop=True)
            m2Tsb = sb.tile([128, GA], BF16, tag="m2Tsb")
            nc.vector.tensor_copy(m2Tsb[:], m2T_ps)

            out_ps = small[:, 0:GA]
            nc.tensor.matmul(out_ps, w["Wsp"][:], m2Tsb[:], start=True, stop=True)
            outsb = sb.tile([128, GA], F32, tag="outsb")
            nc.vector.tensor_copy(outsb[:], out_ps)
            nc.sync.dma_start(d_out[:, bs * A:(bs + g) * A], outsb[:])

    nc.compile()
    return nc


def get_program(**kw):
    key = tuple(sorted(kw.items()))
    if key not in _PROGRAM_CACHE:
        _PROGRAM_CACHE[key] = _build_program(**kw)
    return _PROGRAM_CACHE[key]


def _host_prep(a_token, m_token, a2m_pe, a_pe, Wq, Wk, Wv, W_cross_proj,
               W_cattn, W_self_proj, a2m_relation, a_relation):
    f32 = np.float32
    B = BT

    # dense exp-biases
    safe1 = np.maximum(a2m_relation, 0)[..., None]
    bias1 = np.take_along_axis(a2m_pe, safe1, axis=2)           # [B,A,M,H]
    bias1 = np.where(a2m_relation[..., None] >= 0, bias1, f32(NEG))
    bias1 = bias1.transpose(0, 3, 1, 2).reshape(B, 80, M)       # [B,(h,q),kv]
    eb1 = np.zeros((B, 80, MPAD), dtype=BF)
    eb1[:, :, :M] = np.exp(bias1).astype(BF)
    # -> [128, B, 3, 80]
    eb1 = np.ascontiguousarray(
        eb1.reshape(B, 80, 3, 128).transpose(3, 0, 2, 1))

    safe2 = np.maximum(a_relation, 0)[..., None]
    bias2 = np.take_along_axis(a_pe, safe2, axis=2)
    bias2 = np.where(a_relation[..., None] >= 0, bias2, f32(NEG))
    eb2 = np.exp(bias2.transpose(0, 3, 1, 2).reshape(B, 80, A))
    eb2 = np.ascontiguousarray(eb2.transpose(2, 0, 1)).astype(BF)  # [10, B, 80]

    Wcc = (W_cross_proj.astype(f32) @ W_cattn.astype(f32))
    consts = {
        "Wq4": (Wq.astype(f32) / 4.0).astype(BF),
        "WkT": np.ascontiguousarray(Wk.T).astype(BF),
        "Wv": Wv.astype(BF),
        "Wccq": (Wcc[:, :128] / 4.0).astype(BF),
        "WckT": np.ascontiguousarray(Wcc[:, 128:256].T).astype(BF),
        "Wccv": Wcc[:, 256:384].astype(BF),
        "Wsp": W_self_proj.astype(BF),
        "QselG": np.tile(np.tile(np.eye(A, dtype=BF), (N_HEADS, 1)), (1, G)),
        "I128": np.eye(128, dtype=BF),
    }
    hmask = np.zeros((80, 128), dtype=BF)
    for h in range(N_HEADS):
        hmask[h * A:(h + 1) * A, h * HD:(h + 1) * HD] = 1
    consts["Hmask"] = hmask
    bdmask = np.zeros((128, 80), dtype=BF)
    for h in range(N_HEADS):
        bdmask[h * HD:(h + 1) * HD, h * A:(h + 1) * A] = 1
    consts["BDmask"] = bdmask

    nb = NB
    in_maps = []
    for c in range(N_CORES):
        lo, hi = c * nb, (c + 1) * nb
        im = dict(consts)
        mt = np.zeros((128, nb, MPAD), dtype=BF)
        mt[:, :, :M] = m_token[lo:hi].transpose(2, 0, 1).astype(BF)
        im["mT"] = mt.reshape(128, nb * MPAD)
        im["aT"] = np.ascontiguousarray(
            a_token[lo:hi].transpose(2, 0, 1)).reshape(128, nb * A).astype(BF)
        # eB1 core slice: [128, nb, 3, 80] -> group-reorder [128, ng, 3, g, 80]
        e1 = eb1[:, lo:hi]                                    # [128, nb, 3, 80]
        e1 = e1.reshape(128, nb // G, G, 3, 80).transpose(0, 1, 3, 2, 4)
        im["eB1"] = np.ascontiguousarray(e1).reshape(128, nb * 240)
        im["eB2"] = np.ascontiguousarray(eb2[:, lo:hi]).reshape(10, nb * 80)
        in_maps.append(im)
    return in_maps


def kernel(a_token, m_token, a2m_pe, a_pe, Wq, Wk, Wv, W_cross_proj,
           W_cattn, W_self_proj, a2m_relation, a_relation):
    from concourse.bass_utils import run_bass_kernel_spmd

    nc = get_program()
    in_maps = _host_prep(a_token, m_token, a2m_pe, a_pe, Wq, Wk, Wv,
                         W_cross_proj, W_cattn, W_self_proj,
                         a2m_relation, a_relation)
    res = run_bass_kernel_spmd(nc, in_maps, list(range(N_CORES)))
    nb = NB
    out = np.empty((BT, A, DIM), dtype=np.float32)
    for c in range(N_CORES):
        out[c * nb:(c + 1) * nb] = res.results[c]["outT"].T.reshape(nb, A, DIM)
    return out
